# revision 13
# baseline (speedup 1.0000x reference)
"""Trainium2 Bass kernel for nn_AttnBlock (per-pixel qk attention block).

Reference computation (per batch b):
  q = x @ wq.T ; k = x @ wk.T ; v = x @ wv.T          # [H*W, 512], heads n=8, d=64
  s[n, p]    = sum_d q[p, n*64+d] * k[p, n*64+d]      # per-pixel dot product
  w[n, h, :] = softmax(s[n, h, :] * d**-0.5)          # softmax over W axis (32)
  vsum[n, p] = sum_d v[p, n*64+d]
  out[b, n, hw, xy] = w[n, hw] * vsum[n, xy]          # outer product per batch

Sharding: data-parallel over batch: core b handles batch b (8 cores, B=8).

The kernel is output-write bound: fp16 output (host upcasts; rel err ~1e-3
vs the 2e-2 gate) => 16 MB/core to write + 2 MB of inputs to read at the
~358 GB/s per-NC HBM limit => ~52.7 us of DMA floor. The design keeps one
DMA queue busy end-to-end: inputs stream in (packed into 9 DMAs to amortize
the ~650 ns per-DMA issue cost), and the first output tiles are ready
before the last input lands so the output chain starts immediately after.

v5 implementation notes:
- host does layout prep only: x^T, per-pair-blocked wq/wk, head-block-summed
  wv, packed aux tensors, fp16 casts.
- input order: aux1(wvt+ind2), wq/wk pair-0 slices, x^T channel chunks,
  aux2(sel+ident2), packed pair-1..3 w slices. vsum + pair-0 q/k matmuls
  interleave per channel chunk, trailing the x-chunk DMAs on a warmed-up PE
  (dummy matmuls from t~0 reach full 2.4 GHz clock before x lands).
- q/k PSUM never drains: sprod = q*k reads both PSUM banks directly.
- pair 0 runs a fine-grained softmax (per-128-pixel chunk exp/reduce/recip/
  mult/PE-transpose) so production of the first 1 MB starts ~2 us after the
  last x chunk; pairs 1-3 use half-granularity (512) ops.
- production per head: PE selector-matmul broadcast of vsum, 8 outer-product
  tiles [128, 1024] via DVE tensor_scalar fp16 (4x mode, 327 ns) with 2 on
  ACT for heads 1-7; head 0 is all-DVE and split into 2x 1 MB DMAs.
"""

import numpy as np

import concourse.bass as bass
import concourse.mybir as mybir
import concourse.tile as tile
from concourse import bacc
from concourse.bass_utils import run_bass_kernel_spmd

F32 = mybir.dt.float32
F16 = mybir.dt.float16

B, HW, DIM = 8, 1024, 512
N_HEADS, D_HEAD = 8, 64
N_CORES = 8
SCALE = float(D_HEAD) ** -0.5

QK_DT = F16
QK_NP = np.float16
OUT_DT = F16

N_WARMUP = 8  # dummy PE matmuls (~3 us) to reach full tensor-engine clock


def build_program(loop_iters=None):
    """loop_iters: if set, wrap the whole kernel body in a tc.For_i hardware
    loop (benchmarking only -- one NEFF executes the body N times)."""
    # Bacc (not raw Bass): its compile() runs move_matmul_waits_to_ldweights,
    # without which any matmul with >1 semaphore wait fails walrus codegen.
    nc = bacc.Bacc(None)

    xt_d = nc.declare_dram_parameter("xt", [DIM, HW], QK_DT, isOutput=False)
    # pair-0 slices [p, k, o]; pairs 1-3 packed into one tensor
    wq0_d = nc.declare_dram_parameter("wq0", [128, 4, 128], QK_DT, isOutput=False)
    wk0_d = nc.declare_dram_parameter("wk0", [128, 4, 128], QK_DT, isOutput=False)
    wrest_d = nc.declare_dram_parameter(
        "wrest", [128, 2, 3, 4, 128], QK_DT, isOutput=False
    )
    aux1_d = nc.declare_dram_parameter("aux1", [128, 34], QK_DT, isOutput=False)
    aux2_d = nc.declare_dram_parameter("aux2", [8, 1026], QK_DT, isOutput=False)
    y_d = nc.declare_dram_parameter("y", [N_HEADS, HW, HW], OUT_DT, isOutput=True)

    with tile.TileContext(nc) as tc:
        with (
            tc.tile_pool(name="singles", bufs=1) as singles,
            tc.tile_pool(name="sprod", bufs=2) as sprodp,
            tc.tile_pool(name="smax", bufs=2) as smaxp,
            tc.tile_pool(name="wt", bufs=2) as wtp,
            tc.tile_pool(name="bc", bufs=2) as bcp,
            tc.tile_pool(name="prod", bufs=2) as prodp,
            tc.tile_pool(name="tp_ps", bufs=1, space="PSUM") as tp_ps,
            tc.tile_pool(name="qk_ps", bufs=4, space="PSUM") as qk_ps,
            tc.tile_pool(name="s_ps", bufs=1, space="PSUM") as s_ps,
            tc.tile_pool(name="v_ps", bufs=1, space="PSUM") as v_ps,
        ):
            def emit_body():
                # ---- PE warm-up: dummy matmuls from t~0 --------------------
                wm = singles.tile([128, 512], QK_DT, name="wm")
                nc.gpsimd.memset(wm, 0.0)
                for wi in range(N_WARMUP):
                    wps = tp_ps.tile([128, 512], F32, tag="tp")
                    nc.tensor.matmul(
                        wps, wm[:, 0:128], wm, start=True, stop=True,
                    )

                # ---- loads: one FIFO queue (sync HWDGE), priority order ----
                aux1_sb = singles.tile([128, 34], QK_DT)
                nc.sync.dma_start(out=aux1_sb, in_=aux1_d[:])
                wvt_sb = aux1_sb[:, 0:32].rearrange("p (k n) -> p k n", k=4)
                ind2_sb = aux1_sb[:, 32:34]

                wq_sb = singles.tile([128, 4, 4, 128], QK_DT)  # [p, pair, k, o]
                wk_sb = singles.tile([128, 4, 4, 128], QK_DT)
                nc.sync.dma_start(out=wq_sb[:, 0], in_=wq0_d[:])
                nc.sync.dma_start(out=wk_sb[:, 0], in_=wk0_d[:])

                xTb = singles.tile([128, 4, HW], QK_DT, name="xTb")
                xv = xt_d[:].rearrange("(k p) xy -> p k xy", p=128)
                for ki in range(4):
                    nc.sync.dma_start(out=xTb[:, ki, :], in_=xv[:, ki, :])

                aux2_sb = singles.tile([8, 1026], QK_DT)
                nc.sync.dma_start(out=aux2_sb, in_=aux2_d[:])
                sel_sb = aux2_sb[:, 0:1024]
                ident2_sb = aux2_sb[0:2, 1024:1026]

                nc.sync.dma_start(
                    out=wq_sb[:, 1:4], in_=wrest_d[:, 0],
                )
                nc.sync.dma_start(
                    out=wk_sb[:, 1:4], in_=wrest_d[:, 1],
                )

                # ---- pair 0 q/k + vsum: interleaved per channel chunk ------
                vps = v_ps.tile([N_HEADS, HW], F32)
                vsum_sb = singles.tile([N_HEADS, HW], QK_DT)
                qk0_ps = []
                for nj in range(2):
                    qps0 = qk_ps.tile([128, 512], F32, tag="qk", name=f"q0_{nj}")
                    kps0 = qk_ps.tile([128, 512], F32, tag="qk", name=f"k0_{nj}")
                    qk0_ps.append((qps0, kps0))
                for ki in range(4):
                    for nj in range(2):
                        qps, kps = qk0_ps[nj]
                        xr = xTb[:, ki, nj * 512 : (nj + 1) * 512]
                        nc.tensor.matmul(
                            qps, wq_sb[:, 0, ki, :], xr,
                            start=(ki == 0), stop=(ki == 3),
                        )
                        nc.tensor.matmul(
                            kps, wk_sb[:, 0, ki, :], xr,
                            start=(ki == 0), stop=(ki == 3),
                        )
                        nc.tensor.matmul(
                            vps[:, nj * 512 : (nj + 1) * 512],
                            wvt_sb[:, ki, :], xr,
                            start=(ki == 0), stop=(ki == 3),
                        )
                def emit_vsum_drain(nj):
                    # vsum -> fp16 on DVE (gates the selector broadcast);
                    # emitted into DVE idle slots behind the softmax chain
                    nc.vector.tensor_copy(
                        vsum_sb[:, nj * 512 : (nj + 1) * 512],
                        vps[:, nj * 512 : (nj + 1) * 512],
                    )

                def emit_scores_half(ti, nj, qps, kps):
                    """sprod straight from the q/k PSUM banks + score matmul.
                    Returns the score PSUM tile [2, 512]."""
                    sprod = sprodp.tile([128, 512], QK_DT, tag="sp")
                    nc.vector.tensor_tensor(
                        out=sprod, in0=qps, in1=kps, op=mybir.AluOpType.mult,
                    )
                    sps = s_ps.tile([2, 512], F32, tag="s")
                    nc.tensor.matmul(sps, ind2_sb, sprod, start=True, stop=True)
                    return sps

                def emit_softmax_chunk(sps, c0, c1):
                    """exp/reduce/recip/mult over score columns [c0*128,c1*128).
                    Returns w chunk tile [2, (c1-c0)*128] fp16."""
                    ncol = (c1 - c0) * 128
                    csl = slice(c0 * 128 - (c0 // 4) * 512, c1 * 128 - (c0 // 4) * 512)
                    e_h = smaxp.tile([2, ncol], QK_DT, tag=f"e{c0}")
                    nc.scalar.activation(
                        out=e_h, in_=sps[:, csl],
                        func=mybir.ActivationFunctionType.Exp,
                        scale=SCALE,
                    )
                    nh = ncol // 32
                    denom = smaxp.tile([2, nh], QK_DT, tag=f"d{c0}")
                    with nc.allow_low_precision(reason="fp16 softmax denom"):
                        nc.vector.tensor_reduce(
                            out=denom,
                            in_=e_h.rearrange("p (h w) -> p h w", w=32),
                            axis=mybir.AxisListType.X,
                            op=mybir.AluOpType.add,
                        )
                        rden = smaxp.tile([2, nh], QK_DT, tag=f"r{c0}")
                        nc.vector.reciprocal(rden, denom)
                    w_h = smaxp.tile([2, ncol], QK_DT, tag=f"w{c0}")
                    rden_b = bass.AP(
                        tensor=rden.tensor, offset=rden.offset,
                        ap=[*rden.ap, [0, 32]],
                    )
                    nc.vector.tensor_tensor(
                        out=w_h.rearrange("p (h w) -> p h w", w=32),
                        in0=e_h.rearrange("p (h w) -> p h w", w=32),
                        in1=rden_b,
                        op=mybir.AluOpType.mult,
                    )
                    return w_h

                def emit_bcast(head, dve_drains):
                    """vsum row -> all partitions via PE selector matmul."""
                    bcast_t = bcp.tile([128, HW], QK_DT, tag="bc")
                    for nj in range(2):
                        bps = qk_ps.tile([128, 512], F32, tag="qk")
                        nc.tensor.matmul(
                            bps,
                            sel_sb[:, head * 128 : (head + 1) * 128],
                            vsum_sb[:, nj * 512 : (nj + 1) * 512],
                            start=True, stop=True,
                        )
                        dstb = bcast_t[:, nj * 512 : (nj + 1) * 512]
                        if nj < dve_drains:
                            nc.vector.tensor_copy(dstb, bps)
                        else:
                            nc.scalar.copy(dstb, bps)
                    return bcast_t

                def dma_rows(head, prod_t, j, c0, c1):
                    nc.sync.dma_start(
                        out=y_d[head : head + 1].rearrange(
                            "n (c p) xy -> p n c xy", p=128
                        )[:, :, c0:c1, :],
                        in_=prod_t[:, j : j + 1, c0:c1, :],
                    )

                # ---- pair 0: fine-grained softmax -> earliest first DMA ----
                # scores per half, then per-128-col chunks through softmax and
                # PE transpose; wt drains in two 4-col groups.
                wt_sb = wtp.tile([128, 8, 2], F32, tag="wt")
                tp = tp_ps.tile([128, 16], QK_DT, tag="tp")
                bcast0 = None
                prod0 = prodp.tile([128, 2, 8, HW], OUT_DT, tag="pr")
                for nj in range(2):
                    sps = emit_scores_half(0, nj, *qk0_ps[nj])
                    if nj == 0:
                        emit_vsum_drain(0)
                        emit_vsum_drain(1)
                    for cj in range(nj * 4, nj * 4 + 4):
                        w_c = emit_softmax_chunk(sps, cj, cj + 1)
                        nc.tensor.transpose(
                            tp[:, cj * 2 : (cj + 1) * 2], w_c, ident2_sb,
                        )
                    if nj == 0:
                        # first half of wt + head-0 broadcast, then the first
                        # 4 production tiles and the opening 1 MB DMA
                        nc.vector.tensor_copy(
                            wt_sb[:, 0:4, :],
                            tp[:, 0:8].rearrange("p (c n) -> p c n", c=4),
                        )
                        bcast0 = emit_bcast(0, dve_drains=1)
                        for cj in range(4):
                            nc.vector.tensor_scalar_mul(
                                prod0[:, 0, cj, :], bcast0, wt_sb[:, cj, 0:1],
                            )
                        dma_rows(0, prod0, 0, 0, 4)
                    else:
                        nc.vector.tensor_copy(
                            wt_sb[:, 4:8, :],
                            tp[:, 8:16].rearrange("p (c n) -> p c n", c=4),
                        )
                        for cj in range(4, 8):
                            nc.vector.tensor_scalar_mul(
                                prod0[:, 0, cj, :], bcast0, wt_sb[:, cj, 0:1],
                            )
                        dma_rows(0, prod0, 0, 4, 8)
                # head 1 of pair 0
                bcast1 = emit_bcast(1, dve_drains=1)
                for cj in range(8):
                    if cj in (2, 5):
                        nc.scalar.activation(
                            out=prod0[:, 1, cj, :], in_=bcast1,
                            func=mybir.ActivationFunctionType.Copy,
                            scale=wt_sb[:, cj, 1:2],
                        )
                    else:
                        nc.vector.tensor_scalar_mul(
                            prod0[:, 1, cj, :], bcast1, wt_sb[:, cj, 1:2],
                        )
                dma_rows(1, prod0, 1, 0, 8)

                # ---- pairs 1-3: half-granularity pipeline ------------------
                for ti in range(1, 4):
                    w_halves = []
                    for nj in range(2):
                        qps = qk_ps.tile([128, 512], F32, tag="qk")
                        kps = qk_ps.tile([128, 512], F32, tag="qk")
                        for ps, w_sb in ((qps, wq_sb), (kps, wk_sb)):
                            for ki in range(4):
                                nc.tensor.matmul(
                                    ps,
                                    w_sb[:, ti, ki, :],
                                    xTb[:, ki, nj * 512 : (nj + 1) * 512],
                                    start=(ki == 0), stop=(ki == 3),
                                )
                        sps = emit_scores_half(ti, nj, qps, kps)
                        w_halves.append(emit_softmax_chunk(sps, nj * 4, nj * 4 + 4))
                    wt_sb = wtp.tile([128, 8, 2], F32, tag="wt")
                    tp = tp_ps.tile([128, 16], QK_DT, tag="tp")
                    for cj in range(8):
                        nc.tensor.transpose(
                            tp[:, cj * 2 : (cj + 1) * 2],
                            w_halves[cj // 4][:, (cj % 4) * 128 : (cj % 4 + 1) * 128],
                            ident2_sb,
                        )
                    nc.vector.tensor_copy(
                        wt_sb, tp.rearrange("p (c n) -> p c n", c=8)
                    )
                    prod_t = prodp.tile([128, 2, 8, HW], OUT_DT, tag="pr")
                    for j in range(2):
                        bcast_t = emit_bcast(2 * ti + j, dve_drains=1)
                        for cj in range(8):
                            if cj in (2, 5):
                                nc.scalar.activation(
                                    out=prod_t[:, j, cj, :], in_=bcast_t,
                                    func=mybir.ActivationFunctionType.Copy,
                                    scale=wt_sb[:, cj, j : j + 1],
                                )
                            else:
                                nc.vector.tensor_scalar_mul(
                                    prod_t[:, j, cj, :], bcast_t,
                                    wt_sb[:, cj, j : j + 1],
                                )
                        dma_rows(2 * ti + j, prod_t, j, 0, 8)

            if loop_iters:
                with tc.For_i(0, loop_iters, 1):
                    emit_body()
            else:
                emit_body()

    nc.compile()
    return nc


_NC_CACHE = None


def _get_nc():
    global _NC_CACHE
    if _NC_CACHE is None:
        _NC_CACHE = build_program()
    return _NC_CACHE


def make_in_maps(x, wq, wk, wv):
    """Host-side input prep: dtype casts and layout transforms only (transpose,
    reshape, head-block sum of wv -- no x-dependent compute beyond layout),
    plus per-core batch sharding."""
    x = np.ascontiguousarray(np.asarray(x, dtype=np.float32))
    wq = np.asarray(wq, dtype=np.float32)
    wk = np.asarray(wk, dtype=np.float32)
    wv = np.asarray(wv, dtype=np.float32)
    b, H, W, dim = x.shape
    assert (b, H, W, dim) == (B, 32, 32, DIM)

    # blocked [pair, p, k, o]: wb[t, p, k, o] = w.T[k*128+p, t*128+o]
    def blocked(w):
        wt = np.ascontiguousarray(w.T).astype(QK_NP)        # [c, o]
        return np.ascontiguousarray(
            wt.reshape(4, 128, 4, 128).transpose(2, 1, 0, 3)
        )

    wqb = blocked(wq)
    wkb = blocked(wk)
    # pairs 1-3 packed: [p, {q,k}, t-1, k, o]
    wrest = np.ascontiguousarray(
        np.stack([wqb[1:4], wkb[1:4]], axis=0).transpose(2, 0, 1, 3, 4)
    )
    wvt = np.ascontiguousarray(
        wv.reshape(N_HEADS, D_HEAD, DIM).sum(axis=1).T     # [c, n]
    ).astype(QK_NP)
    ind2 = np.zeros((128, 2), dtype=QK_NP)
    ind2[np.arange(128), np.arange(128) // D_HEAD] = 1.0
    aux1 = np.concatenate([wvt.reshape(4, 128, 8).transpose(1, 0, 2)
                           .reshape(128, 32), ind2], axis=1)
    sel = np.zeros((N_HEADS, N_HEADS * 128), dtype=QK_NP)
    for n in range(N_HEADS):
        sel[n, n * 128 : (n + 1) * 128] = 1.0
    aux2 = np.zeros((8, 1026), dtype=QK_NP)
    aux2[:, 0:1024] = sel
    aux2[0:2, 1024:1026] = np.eye(2, dtype=QK_NP)

    xh = x.reshape(B, HW, DIM).astype(QK_NP)
    return [
        {
            "xt": np.ascontiguousarray(xh[i].T),           # [c, xy]
            "wq0": wqb[0],
            "wk0": wkb[0],
            "wrest": wrest,
            "aux1": np.ascontiguousarray(aux1),
            "aux2": aux2,
        }
        for i in range(N_CORES)
    ]


def kernel(x, wq, wk, wv):
    nc = _get_nc()
    in_maps = make_in_maps(x, wq, wk, wv)
    res = run_bass_kernel_spmd(nc, in_maps, list(range(N_CORES)))
    out = np.stack([res.results[i]["y"] for i in range(N_CORES)], axis=0)
    # [b, n, hw, xy] -> [b, n, h, w, x, y]; upcast fp16 -> fp32 on host
    return out.astype(np.float32).reshape(B, N_HEADS, 32, 32, 32, 32)


if __name__ == "__main__":
    rng = np.random.default_rng(0)
    x = rng.standard_normal((B, 32, 32, DIM), dtype=np.float32)
    s = 1.0 / np.sqrt(512.0)
    wq = rng.uniform(-s, s, (512, 512)).astype(np.float32)
    wk = rng.uniform(-s, s, (512, 512)).astype(np.float32)
    wv = rng.uniform(-s, s, (512, 512)).astype(np.float32)
    y = kernel(x=x, wq=wq, wk=wk, wv=wv)
    print(y.shape, y.dtype)


# revision 17
# speedup vs baseline: 1.0518x; 1.0518x over previous
"""Trainium2 Bass kernel for nn_AttnBlock (per-pixel qk attention block).

Reference computation (per batch b):
  q = x @ wq.T ; k = x @ wk.T ; v = x @ wv.T          # [H*W, 512], heads n=8, d=64
  s[n, p]    = sum_d q[p, n*64+d] * k[p, n*64+d]      # per-pixel dot product
  w[n, h, :] = softmax(s[n, h, :] * d**-0.5)          # softmax over W axis (32)
  vsum[n, p] = sum_d v[p, n*64+d]
  out[b, n, hw, xy] = w[n, hw] * vsum[n, xy]          # outer product per batch

Sharding: data-parallel over batch: core b handles batch b (8 cores, B=8).

The kernel is output-write bound: fp16 output (host upcasts; rel err ~1e-3
vs the 2e-2 gate) => 16 MB/core written + 2 MB read at the ~358 GB/s per-NC
HBM limit => ~52.7 us DMA floor. The design keeps the DMA queue busy
end-to-end: the input stream is packed into 7 DMAs (aux+pair-0 weights
merged into one "pre" tensor so x^T starts immediately), and the first
output tiles are ready shortly after the last input lands.

v6 implementation notes:
- host does layout prep only (transposes/reshapes/casts; the only
  arithmetic is the head-block sum of wv rows, O(dim^2)).
- PE warm-up: dummy matmuls (own PSUM bank) from t~0 so the tensor engine
  reaches full clock (cost model: 2.4 GHz after 3 us continuous busy)
  before the x-gated burst; spares also fill x-chunk stall gaps.
- q/k PSUM never drains: sprod = q*k reads both PSUM banks directly.
- pair 0 runs a fine-grained high-priority softmax (per-128-col chunk
  exp/reduce/recip/mult/PE-transpose) so the first 1 MB DMA starts ~5 us
  after the last x chunk; pairs 1-3 use half-granularity ops.
- production per head: PE selector-matmul broadcast of vsum (drains on
  ACT), 8 outer-product tiles [128, 1024] via DVE tensor_scalar fp16
  (4x mode, 327 ns; 2 of 8 on ACT for heads 1-7), then the head's 2 MB
  DMA (head 0: 2x 1 MB to open the chain early).
"""

import numpy as np

import concourse.bass as bass
import concourse.mybir as mybir
import concourse.tile as tile
from concourse import bacc
from concourse.bass_utils import run_bass_kernel_spmd

F32 = mybir.dt.float32
F16 = mybir.dt.float16

B, HW, DIM = 8, 1024, 512
N_HEADS, D_HEAD = 8, 64
N_CORES = 8
SCALE = float(D_HEAD) ** -0.5

QK_DT = F16
QK_NP = np.float16
OUT_DT = F16

N_WARMUP = 16  # dummy PE matmuls: ~3 us ramp + spares for x-stall gaps


def build_program(loop_iters=None):
    """loop_iters: if set, wrap the whole kernel body in a tc.For_i hardware
    loop (benchmarking only -- one NEFF executes the body N times)."""
    # Bacc (not raw Bass): its compile() runs move_matmul_waits_to_ldweights,
    # without which any matmul with >1 semaphore wait fails walrus codegen.
    nc = bacc.Bacc(None)

    xt_d = nc.declare_dram_parameter("xt", [DIM, HW], QK_DT, isOutput=False)
    # pre = aux (wv_sum + ind2) and the pair-0 wq/wk slices, one DMA
    pre_d = nc.declare_dram_parameter("pre", [128, 1058], QK_DT, isOutput=False)
    wrest_d = nc.declare_dram_parameter(
        "wrest", [128, 2, 3, 4, 128], QK_DT, isOutput=False
    )
    aux2_d = nc.declare_dram_parameter("aux2", [8, 1026], QK_DT, isOutput=False)
    y_d = nc.declare_dram_parameter("y", [N_HEADS, HW, HW], OUT_DT, isOutput=True)

    with tile.TileContext(nc) as tc:
        with (
            tc.tile_pool(name="singles", bufs=1) as singles,
            tc.tile_pool(name="sprod", bufs=2) as sprodp,
            tc.tile_pool(name="smax", bufs=2) as smaxp,
            tc.tile_pool(name="wt", bufs=2) as wtp,
            tc.tile_pool(name="bc", bufs=2) as bcp,
            tc.tile_pool(name="prod", bufs=2) as prodp,
            tc.tile_pool(name="warm_ps", bufs=1, space="PSUM") as warm_ps,
            tc.tile_pool(name="tp_ps", bufs=1, space="PSUM") as tp_ps,
            tc.tile_pool(name="qk_ps", bufs=3, space="PSUM") as qk_ps,
            tc.tile_pool(name="s_ps", bufs=1, space="PSUM") as s_ps,
            tc.tile_pool(name="v_ps", bufs=1, space="PSUM") as v_ps,
        ):
            def emit_body():
                # ---- PE warm-up tile (dummy matmuls emitted last, so they
                # have the lowest priority and only fill idle PE slots) ------
                wm = singles.tile([128, 512], QK_DT, name="wm")
                nc.gpsimd.memset(wm, 0.0)

                # ---- loads: one FIFO queue (sync HWDGE), priority order ----
                pre_sb = singles.tile([128, 1058], QK_DT)
                nc.sync.dma_start(out=pre_sb, in_=pre_d[:])
                wvt_sb = pre_sb[:, 0:32].rearrange("p (k n) -> p k n", k=4)
                ind2_sb = pre_sb[:, 32:34]
                wq0_sb = pre_sb[:, 34:546].rearrange("p (k o) -> p k o", k=4)
                wk0_sb = pre_sb[:, 546:1058].rearrange("p (k o) -> p k o", k=4)

                xT = []
                xv = xt_d[:].rearrange("(k p) xy -> p k xy", p=128)
                for ki in range(4):
                    xt_t = singles.tile([128, HW], QK_DT, name=f"xT{ki}")
                    nc.sync.dma_start(out=xt_t, in_=xv[:, ki, :])
                    xT.append(xt_t)

                aux2_sb = singles.tile([8, 1026], QK_DT)
                nc.sync.dma_start(out=aux2_sb, in_=aux2_d[:])
                sel_sb = aux2_sb[:, 0:1024]
                ident2_sb = aux2_sb[0:2, 1024:1026]

                wq_sb = singles.tile([128, 3, 4, 128], QK_DT)  # pairs 1-3
                wk_sb = singles.tile([128, 3, 4, 128], QK_DT)
                nc.sync.dma_start(out=wq_sb, in_=wrest_d[:, 0])
                nc.sync.dma_start(out=wk_sb, in_=wrest_d[:, 1])

                def wslice(w0_sb, wr_sb, ti, ki):
                    if ti == 0:
                        return w0_sb[:, ki, :]
                    return wr_sb[:, ti - 1, ki, :]

                def emit_qk_half(ti, nj):
                    """q/k matmul groups for pixel-half nj of pair ti."""
                    qps = qk_ps.tile([128, 512], F32, tag="qk", name="qps")
                    kps = qk_ps.tile([128, 512], F32, tag="qk", name="kps")
                    for ps, w0, wr in ((qps, wq0_sb, wq_sb), (kps, wk0_sb, wk_sb)):
                        for ki in range(4):
                            nc.tensor.matmul(
                                ps,
                                wslice(w0, wr, ti, ki),
                                xT[ki][:, nj * 512 : (nj + 1) * 512],
                                start=(ki == 0),
                                stop=(ki == 3),
                            )
                    return qps, kps

                def emit_scores_half(qps, kps):
                    """sprod straight from the q/k PSUM banks + score matmul."""
                    sprod = sprodp.tile([128, 512], QK_DT, tag="sp")
                    nc.vector.tensor_tensor(
                        out=sprod, in0=qps, in1=kps, op=mybir.AluOpType.mult,
                    )
                    sps = s_ps.tile([2, 512], F32, tag="s")
                    nc.tensor.matmul(sps, ind2_sb, sprod, start=True, stop=True)
                    return sps

                def emit_softmax_chunk(sps, cloc, ncol):
                    """exp/reduce/recip/mult over sps cols [cloc, cloc+ncol).
                    Returns the w chunk tile [2, ncol] fp16."""
                    csl = slice(cloc, cloc + ncol)
                    e_h = smaxp.tile([2, ncol], QK_DT, tag=f"e{cloc}_{ncol}")
                    nc.scalar.activation(
                        out=e_h, in_=sps[:, csl],
                        func=mybir.ActivationFunctionType.Exp,
                        scale=SCALE,
                    )
                    nh = ncol // 32
                    denom = smaxp.tile([2, nh], QK_DT, tag=f"d{cloc}_{ncol}")
                    with nc.allow_low_precision(reason="fp16 softmax denom"):
                        nc.vector.tensor_reduce(
                            out=denom,
                            in_=e_h.rearrange("p (h w) -> p h w", w=32),
                            axis=mybir.AxisListType.X,
                            op=mybir.AluOpType.add,
                        )
                        rden = smaxp.tile([2, nh], QK_DT, tag=f"r{cloc}_{ncol}")
                        nc.vector.reciprocal(rden, denom)
                    w_h = smaxp.tile([2, ncol], QK_DT, tag=f"w{cloc}_{ncol}")
                    rden_b = bass.AP(
                        tensor=rden.tensor, offset=rden.offset,
                        ap=[*rden.ap, [0, 32]],
                    )
                    nc.vector.tensor_tensor(
                        out=w_h.rearrange("p (h w) -> p h w", w=32),
                        in0=e_h.rearrange("p (h w) -> p h w", w=32),
                        in1=rden_b,
                        op=mybir.AluOpType.mult,
                    )
                    return w_h

                def emit_bcast(head, bcast_t=None):
                    """vsum row -> all partitions via PE selector matmul;
                    PSUM drains on ACT."""
                    if bcast_t is None:
                        bcast_t = bcp.tile([128, HW], QK_DT, tag="bc", name="bc")
                    for nj in range(2):
                        bps = qk_ps.tile([128, 512], F32, tag="qk", name="bps")
                        nc.tensor.matmul(
                            bps,
                            sel_sb[:, head * 128 : (head + 1) * 128],
                            vsum_sb[:, nj * 512 : (nj + 1) * 512],
                            start=True, stop=True,
                        )
                        nc.scalar.copy(bcast_t[:, nj * 512 : (nj + 1) * 512], bps)
                    return bcast_t

                def dma_rows(head, prod_t, j, c0, c1):
                    nc.sync.dma_start(
                        out=y_d[head : head + 1].rearrange(
                            "n (c p) xy -> p n c xy", p=128
                        )[:, :, c0:c1, :],
                        in_=prod_t[:, j : j + 1, c0:c1, :],
                    )

                # ---- pair 0 first: its q/k + scores get high priority so
                # the first output DMA lands as early as possible; vsum sits
                # between pair-0 h0 and h1 on the PE. All pairs share the
                # same half-granularity softmax/production pipeline.
                with tc.high_priority():
                    qk00 = emit_qk_half(0, 0)

                vps = v_ps.tile([N_HEADS, HW], F32)
                vsum_sb = singles.tile([N_HEADS, HW], QK_DT)
                for nj in range(2):
                    for ki in range(4):
                        nc.tensor.matmul(
                            vps[:, nj * 512 : (nj + 1) * 512],
                            wvt_sb[:, ki, :],
                            xT[ki][:, nj * 512 : (nj + 1) * 512],
                            start=(ki == 0),
                            stop=(ki == 3),
                        )

                with tc.high_priority():
                    sps00 = emit_scores_half(*qk00)
                    w_h00 = emit_softmax_chunk(sps00, 0, 512)

                # vsum -> fp16 halves (ACT; gates the selector broadcast)
                for nj in range(2):
                    nc.scalar.copy(
                        vsum_sb[:, nj * 512 : (nj + 1) * 512],
                        vps[:, nj * 512 : (nj + 1) * 512],
                    )

                for ti in range(4):
                    if ti == 0:
                        w_halves = [w_h00]
                        with tc.high_priority():
                            qps, kps = emit_qk_half(0, 1)
                            sps = emit_scores_half(qps, kps)
                            w_halves.append(emit_softmax_chunk(sps, 0, 512))
                    else:
                        w_halves = []
                        for nj in range(2):
                            qps, kps = emit_qk_half(ti, nj)
                            sps = emit_scores_half(qps, kps)
                            w_halves.append(emit_softmax_chunk(sps, 0, 512))
                    wt_sb = wtp.tile([128, 8, 2], F32, tag="wt", name="wt")
                    tp = tp_ps.tile([128, 16], QK_DT, tag="tp", name="tp")
                    for cj in range(8):
                        nc.tensor.transpose(
                            tp[:, cj * 2 : (cj + 1) * 2],
                            w_halves[cj // 4][:, (cj % 4) * 128 : (cj % 4 + 1) * 128],
                            ident2_sb,
                        )
                    nc.vector.tensor_copy(
                        wt_sb, tp.rearrange("p (c n) -> p c n", c=8)
                    )
                    prod_t = prodp.tile([128, 2, 8, HW], OUT_DT, tag="pr", name="pr")
                    for j in range(2):
                        head = 2 * ti + j
                        bcast_t = emit_bcast(head)
                        for cj in range(8):
                            if cj in (2, 5) and head > 0:
                                nc.scalar.activation(
                                    out=prod_t[:, j, cj, :], in_=bcast_t,
                                    func=mybir.ActivationFunctionType.Copy,
                                    scale=wt_sb[:, cj, j : j + 1],
                                )
                            else:
                                nc.vector.tensor_scalar_mul(
                                    prod_t[:, j, cj, :], bcast_t,
                                    wt_sb[:, cj, j : j + 1],
                                )
                            if head == 0 and cj == 3:
                                dma_rows(0, prod_t, 0, 0, 4)
                        if head == 0:
                            dma_rows(0, prod_t, 0, 4, 8)
                        else:
                            dma_rows(head, prod_t, j, 0, 8)

                # PE warm-up dummies: emitted last => lowest priority, they
                # only run when no real matmul is ready (t~0 and x-stalls)
                for wi in range(N_WARMUP):
                    wps = warm_ps.tile([128, 512], F32, tag="w")
                    nc.tensor.matmul(
                        wps, wm[:, 0:128], wm, start=True, stop=True,
                    )

            if loop_iters:
                with tc.For_i(0, loop_iters, 1):
                    emit_body()
            else:
                emit_body()

    nc.compile()
    return nc


_NC_CACHE = None


def _get_nc():
    global _NC_CACHE
    if _NC_CACHE is None:
        _NC_CACHE = build_program()
    return _NC_CACHE


def make_in_maps(x, wq, wk, wv):
    """Host-side input prep: dtype casts and layout transforms only (transpose,
    reshape, head-block sum of wv -- no x-dependent compute beyond layout),
    plus per-core batch sharding."""
    x = np.ascontiguousarray(np.asarray(x, dtype=np.float32))
    wq = np.asarray(wq, dtype=np.float32)
    wk = np.asarray(wk, dtype=np.float32)
    wv = np.asarray(wv, dtype=np.float32)
    b, H, W, dim = x.shape
    assert (b, H, W, dim) == (B, 32, 32, DIM)

    # blocked [pair, p, k, o]: wb[t, p, k, o] = w.T[k*128+p, t*128+o]
    def blocked(w):
        wt = np.ascontiguousarray(w.T).astype(QK_NP)        # [c, o]
        return np.ascontiguousarray(
            wt.reshape(4, 128, 4, 128).transpose(2, 1, 0, 3)
        )

    wqb = blocked(wq)
    wkb = blocked(wk)
    # pairs 1-3 packed: [p, {q,k}, t-1, k, o]
    wrest = np.ascontiguousarray(
        np.stack([wqb[1:4], wkb[1:4]], axis=0).transpose(2, 0, 1, 3, 4)
    )
    wvt = np.ascontiguousarray(
        wv.reshape(N_HEADS, D_HEAD, DIM).sum(axis=1).T     # [c, n]
    ).astype(QK_NP)
    ind2 = np.zeros((128, 2), dtype=QK_NP)
    ind2[np.arange(128), np.arange(128) // D_HEAD] = 1.0
    pre = np.concatenate(
        [
            wvt.reshape(4, 128, 8).transpose(1, 0, 2).reshape(128, 32),
            ind2,
            wqb[0].reshape(128, 512),
            wkb[0].reshape(128, 512),
        ],
        axis=1,
    )
    sel = np.zeros((N_HEADS, N_HEADS * 128), dtype=QK_NP)
    for n in range(N_HEADS):
        sel[n, n * 128 : (n + 1) * 128] = 1.0
    aux2 = np.zeros((8, 1026), dtype=QK_NP)
    aux2[:, 0:1024] = sel
    aux2[0:2, 1024:1026] = np.eye(2, dtype=QK_NP)

    xh = x.reshape(B, HW, DIM).astype(QK_NP)
    return [
        {
            "xt": np.ascontiguousarray(xh[i].T),           # [c, xy]
            "pre": np.ascontiguousarray(pre),
            "wrest": wrest,
            "aux2": aux2,
        }
        for i in range(N_CORES)
    ]


def kernel(x, wq, wk, wv):
    nc = _get_nc()
    in_maps = make_in_maps(x, wq, wk, wv)
    res = run_bass_kernel_spmd(nc, in_maps, list(range(N_CORES)))
    out = np.stack([res.results[i]["y"] for i in range(N_CORES)], axis=0)
    # [b, n, hw, xy] -> [b, n, h, w, x, y]; upcast fp16 -> fp32 on host
    return out.astype(np.float32).reshape(B, N_HEADS, 32, 32, 32, 32)


if __name__ == "__main__":
    rng = np.random.default_rng(0)
    x = rng.standard_normal((B, 32, 32, DIM), dtype=np.float32)
    s = 1.0 / np.sqrt(512.0)
    wq = rng.uniform(-s, s, (512, 512)).astype(np.float32)
    wk = rng.uniform(-s, s, (512, 512)).astype(np.float32)
    wv = rng.uniform(-s, s, (512, 512)).astype(np.float32)
    y = kernel(x=x, wq=wq, wk=wk, wv=wv)
    print(y.shape, y.dtype)
